# revision 33
# baseline (speedup 1.0000x reference)
"""Trainium2 Bass kernel for a dense pre-LN transformer block.

Shapes (hardcoded): B=2, T=2048, C=768, H=12, D=64, hidden=3072, fp32 I/O.

Strategy (8 NeuronCores, two SPMD launches, no collectives):
  Launch 1 (attention): core = (batch b in {0,1}) x (head-group of 3 heads).
    Each core: LN1 (gain/bias folded into weights on host) -> transpose
    activations to feature-major bf16 -> Q/K/V projections for its 3 heads
    -> causal attention computed in S^T = K @ Q^T layout (keys on
    partitions, so the softmax matrix is consumed by the A@V matmul as the
    stationary operand directly -- no transposes of the 2048x2048 matrix).
    Softmax uses no max-subtraction (scores ~ N(0, 0.3); exp is safe) and
    gets its denominator for free from a ones-column appended to V.
    Output: per-head attention output [T, 3*64], token-major.
  Host glue: assemble heads, add V-bias term (softmax weights sum to 1, so
    the LN1-bias-through-Wv term is a per-feature constant), residual add.
  Launch 2 (MLP): core = 512 contiguous tokens of the flattened [4096, C].
    Each core: LN2 (folded) -> transpose -> MLP1 (bf16, relu+bias on
    ScalarE) -> MLP2 -> transpose back -> residual -> output rows.

All matmuls run in bf16 (4x faster than fp32 on the PE; fp32 accumulate in
PSUM). Weights are cast/folded on the host.
"""

import os
import sys
import math

for _p in ("/opt/trn_rl_repo", "/root/.axon_site/_ro/trn_rl_repo"):
    if _p not in sys.path and os.path.isdir(_p):
        sys.path.insert(0, _p)

import numpy as np
import ml_dtypes

import concourse.bass as bass
import concourse.mybir as mybir
import concourse.tile as tile
from concourse import bacc
from concourse import bass_utils
from concourse.masks import make_identity

BF16 = mybir.dt.bfloat16
F32 = mybir.dt.float32
AF = mybir.ActivationFunctionType

B, T, C, H, D = 2, 2048, 768, 12, 64
HID = 4 * C                     # 3072
EPS = 1e-5
SCALE = 1.0 / math.sqrt(C)      # reference scales scores by 1/sqrt(C)
NC_PER_B = 4                    # cores per batch in launch 1
HG = H // NC_PER_B              # heads per core (3)
P = 128
CCH = C // P                    # 6 feature chunks
TBLK = T // P                   # 16 token blocks of 128
NQB = T // 512                  # 4 q-blocks of 512
ROWS2 = (B * T) // 8            # 512 tokens per core in launch 2
HCH = HID // P                  # 24 hidden chunks

_cache = {}


def _ln_block(nc, pool, x_blk, eps_t, rows=P):
    """bn stats over free dim (C=768 via 3x256 subgroups) -> (mean, rstd)."""
    xg = x_blk.rearrange("p (s f) -> p s f", f=256)
    stats = pool.tile([P, 3, 6], F32, tag="ln_stats")
    for s in range(3):
        nc.vector.bn_stats(out=stats[:rows, s, :], in_=xg[:rows, s, :])
    mv = pool.tile([P, 2], F32, tag="ln_mv")
    nc.vector.bn_aggr(out=mv[:rows], in_=stats[:rows])
    rstd = pool.tile([P, 1], F32, tag="ln_rstd")
    nc.scalar.activation(rstd[:rows], mv[:rows, 1:2], AF.Sqrt,
                         bias=eps_t[:rows])
    nc.vector.reciprocal(rstd[:rows], rstd[:rows])
    return mv[:, 0:1], rstd


def build_kernel1():
    """LN1 + QKV (3 heads) + causal attention. One program, SPMD over 8."""
    nc = bacc.Bacc("TRN2", target_bir_lowering=False, debug=False,
                   num_devices=8)
    xb = nc.dram_tensor("xb", [T, C], F32, kind="ExternalInput")
    wq = nc.dram_tensor("wq", [C, HG * D], BF16, kind="ExternalInput")
    wk = nc.dram_tensor("wk", [C, HG * D], BF16, kind="ExternalInput")
    wv = nc.dram_tensor("wv", [C, HG * D], BF16, kind="ExternalInput")
    bq = nc.dram_tensor("bq", [P, 2], F32, kind="ExternalInput")
    bk = nc.dram_tensor("bk", [P, 2], F32, kind="ExternalInput")
    oO = nc.dram_tensor("oO", [T, HG * D], F32, kind="ExternalOutput")

    with tile.TileContext(nc) as tc:
        with (
            tc.tile_pool(name="persist", bufs=1) as pers,
            tc.tile_pool(name="stream", bufs=3) as stream,
            tc.tile_pool(name="small", bufs=4) as small,
            tc.tile_pool(name="pp", bufs=26) as pp,
            tc.tile_pool(name="psb", bufs=1, space="PSUM") as psb,
            tc.tile_pool(name="pss", bufs=3, space="PSUM") as pss,
            tc.tile_pool(name="pst", bufs=2, space="PSUM") as pst,
            tc.tile_pool(name="pso", bufs=2, space="PSUM") as pso,
        ):
            ident = pers.tile([P, P], BF16)
            make_identity(nc, ident)
            # causal mask for a diagonal 128x128 block of S^T[k, q]:
            # keep where q >= k
            mdiag = pers.tile([P, P], BF16)
            nc.gpsimd.memset(mdiag, 1.0)
            nc.gpsimd.affine_select(
                out=mdiag, in_=mdiag, compare_op=mybir.AluOpType.is_ge,
                fill=0.0, base=0, pattern=[[1, P]], channel_multiplier=-1)

            eps_t = pers.tile([P, 1], F32)
            nc.vector.memset(eps_t, EPS)

            wq_t = pers.tile([P, CCH, HG * D], BF16)
            wk_t = pers.tile([P, CCH, HG * D], BF16)
            wv_t = pers.tile([P, CCH, HG * D], BF16)
            nc.sync.dma_start(wq_t, wq.rearrange("(c p) d -> p c d", p=P))
            nc.sync.dma_start(wk_t, wk.rearrange("(c p) d -> p c d", p=P))
            nc.sync.dma_start(wv_t, wv.rearrange("(c p) d -> p c d", p=P))
            bq_t = pers.tile([P, 2], F32)
            bk_t = pers.tile([P, 2], F32)
            nc.sync.dma_start(bq_t, bq[:, :])
            nc.sync.dma_start(bk_t, bk[:, :])

            xhatT_l = [pers.tile([P, CCH, 512], BF16, name=f"xhT_{t}")
                       for t in range(T // 512)]
            x_tiled = xb.rearrange("(o p) c -> p o c", p=P)
            # ---- LN1 + transpose to feature-major ----
            for o in range(TBLK):
                x_blk = stream.tile([P, C], F32, tag="x_blk")
                nc.sync.dma_start(x_blk, x_tiled[:, o, :])
                mean, rstd = _ln_block(nc, small, x_blk, eps_t)
                xhat = stream.tile([P, C], BF16, tag="xhat")
                nc.vector.tensor_scalar(
                    out=xhat, in0=x_blk, scalar1=mean, scalar2=rstd,
                    op0=mybir.AluOpType.subtract, op1=mybir.AluOpType.mult)
                for c in range(CCH):
                    tp = pst.tile([P, P], BF16, tag="tp")
                    nc.tensor.transpose(tp, xhat[:, c * P:(c + 1) * P], ident)
                    nc.any.tensor_copy(
                        xhatT_l[o // 4][:, c, (o % 4) * P:(o % 4 + 1) * P],
                        tp)

            # ---- per 512-token chunk: QK proj, V proj, then attention for
            # q-block tch (causality: q-block tch only needs K/V <= tch), so
            # the PE projection work and ScalarE exp work interleave ----
            QT_l = [pers.tile([P, 2, 512], BF16, name=f"qt_{t}")
                    for t in range(T // 512)]
            KT_l = [pers.tile([P, 2, 512], BF16, name=f"kt_{t}")
                    for t in range(T // 512)]
            vaug_l = [pers.tile([P, HG, D + 1], BF16, name=f"va_{o}")
                      for o in range(TBLK)]
            oout = pers.tile([P, TBLK, HG, D], F32)
            for tch in range(T // 512):
                for dst_l, w_t, b_t in ((QT_l, wq_t, bq_t), (KT_l, wk_t, bk_t)):
                    for slot in range(2):
                        pr = P if slot == 0 else D  # partitions used
                        acc = psb.tile([P, 512], F32, tag="big")
                        for c in range(CCH):
                            nc.tensor.matmul(
                                acc[:pr],
                                w_t[:, c, slot * P: slot * P + pr],
                                xhatT_l[tch][:, c, :],
                                start=(c == 0), stop=(c == CCH - 1))
                        nc.vector.tensor_scalar_add(
                            dst_l[tch][:pr, slot, :],
                            acc[:pr], b_t[:pr, slot:slot + 1])
                for o in range(4 * tch, 4 * tch + 4):
                    nc.gpsimd.memset(vaug_l[o][:, :, D:D + 1], 1.0)
                    acc = psb.tile([P, 512], F32, tag="big")
                    for c in range(CCH):
                        nc.tensor.matmul(
                            acc[:, :HG * D],
                            xhatT_l[o // 4][:, c, (o % 4) * P:(o % 4 + 1) * P],
                            wv_t[:, c, :],
                            start=(c == 0), stop=(c == CCH - 1))
                    nc.vector.tensor_copy(
                        vaug_l[o][:, :, 0:D],
                        acc[:, :HG * D].rearrange("p (h d) -> p h d", h=HG))
                qb = tch
                for h in range(HG):
                    hslot = 0 if h < 2 else 1
                    hbase = D if h == 1 else 0
                    ptiles = {}
                    for kb in range(4 * qb + 4):
                        qs_rel = max(0, kb - 4 * qb) * P
                        n = 512 - qs_rel
                        sc = pss.tile([P, 512], F32, tag="sc")
                        nc.tensor.matmul(
                            sc[:, :n],
                            KT_l[kb // 4][hbase:hbase + D, hslot,
                                          (kb % 4) * P:(kb % 4 + 1) * P],
                            QT_l[qb][hbase:hbase + D, hslot, qs_rel:],
                            start=True, stop=True)
                        pt = pp.tile([P, 512], BF16, tag="p")
                        nc.scalar.activation(pt[:, :n], sc[:, :n], AF.Exp,
                                             scale=SCALE)
                        if kb >= 4 * qb:  # diagonal block: triangular mask
                            nc.vector.tensor_mul(pt[:, 0:P], pt[:, 0:P],
                                                 mdiag)
                        ptiles[kb] = (pt, qs_rel)
                    for s in range(4):
                        g = 4 * qb + s          # global 128-token q index
                        oacc = pso.tile([P, D + 1], F32, tag="oav")
                        for kb in range(g + 1):
                            pt, qs_rel = ptiles[kb]
                            off = s * P - qs_rel
                            nc.tensor.matmul(
                                oacc, pt[:, off:off + P],
                                vaug_l[kb][:, h, :],
                                start=(kb == 0), stop=(kb == g))
                        rec = small.tile([P, 1], F32, tag="rec")
                        nc.vector.reciprocal(rec, oacc[:, D:D + 1])
                        nc.vector.tensor_scalar_mul(
                            oout[:, g, h, :], oacc[:, 0:D], rec)

            nc.sync.dma_start(
                oO.rearrange("(o p) (h d) -> p o h d", p=P, h=HG), oout)
    nc.compile()
    return nc


def build_kernel2():
    """x_mid = x + attn; LN2 + MLP + residual for 512 tokens per core."""
    nc = bacc.Bacc("TRN2", target_bir_lowering=False, debug=False,
                   num_devices=8)
    xq = nc.dram_tensor("xq", [ROWS2, C], F32, kind="ExternalInput")
    aq = nc.dram_tensor("aq", [ROWS2, C], F32, kind="ExternalInput")
    wh = nc.dram_tensor("wh", [C, HID], BF16, kind="ExternalInput")
    wp = nc.dram_tensor("wp", [HID, C], BF16, kind="ExternalInput")
    bh = nc.dram_tensor("bh", [P, HCH], F32, kind="ExternalInput")
    bp = nc.dram_tensor("bp", [P, CCH], F32, kind="ExternalInput")
    oq = nc.dram_tensor("oq", [ROWS2, C], F32, kind="ExternalOutput")

    NO = ROWS2 // P  # 4 token sub-blocks
    with tile.TileContext(nc) as tc:
        with (
            tc.tile_pool(name="persist", bufs=1) as pers,
            tc.tile_pool(name="stream", bufs=3) as stream,
            tc.tile_pool(name="small", bufs=4) as small,
        ):
            eps_t = pers.tile([P, 1], F32)
            nc.vector.memset(eps_t, EPS)
            identb = pers.tile([P, P], BF16)
            make_identity(nc, identb)
            identf = pers.tile([P, P], F32)
            make_identity(nc, identf)

            xm = pers.tile([P, NO, C], F32)        # x_mid, token-major
            x4 = pers.tile([P, NO, C], F32)
            nc.sync.dma_start(x4, xq.rearrange("(o p) c -> p o c", p=P))
            a4 = pers.tile([P, NO, C], F32)
            nc.sync.dma_start(a4, aq.rearrange("(o p) c -> p o c", p=P))
            nc.vector.tensor_add(out=xm, in0=x4, in1=a4)

            wh_t = pers.tile([P, CCH, HID], BF16)
            wp_t = pers.tile([P, HCH, C], BF16)
            wh_r = wh.rearrange("(c p) n -> p c n", p=P)
            for g in range(4):
                nc.sync.dma_start(wh_t[:, :, g * 768:(g + 1) * 768],
                                  wh_r[:, :, g * 768:(g + 1) * 768])
            wp_r = wp.rearrange("(c p) n -> p c n", p=P)
            nc.scalar.dma_start(wp_t[:, :HCH // 2], wp_r[:, :HCH // 2])
            nc.scalar.dma_start(wp_t[:, HCH // 2:], wp_r[:, HCH // 2:])
            bh_t = pers.tile([P, HCH], F32)
            bp_t = pers.tile([P, CCH], F32)
            nc.scalar.dma_start(bh_t, bh[:, :])
            nc.scalar.dma_start(bp_t, bp[:, :])

            # LN2 + transpose -> feature-major bf16 [C, 512]
            xln2T = pers.tile([P, CCH, ROWS2], BF16)
            pst_cm = tc.tile_pool(name="pst", bufs=2, space="PSUM")
            pst = pst_cm.__enter__()
            for o in range(NO):
                mean, rstd = _ln_block(nc, small, xm[:, o, :], eps_t)
                xln = stream.tile([P, C], BF16, tag="xln")
                nc.vector.tensor_scalar(
                    out=xln, in0=xm[:, o, :], scalar1=mean, scalar2=rstd,
                    op0=mybir.AluOpType.subtract, op1=mybir.AluOpType.mult)
                for c in range(CCH):
                    tp = pst.tile([P, P], BF16, tag="tpb")
                    nc.tensor.transpose(tp, xln[:, c * P:(c + 1) * P], identb)
                    nc.vector.tensor_copy(xln2T[:, c, o * P:(o + 1) * P], tp)

            pst_cm.__exit__(None, None, None)
            psb_cm = tc.tile_pool(name="psb", bufs=3, space="PSUM")
            psb = psb_cm.__enter__()
            hidT = pers.tile([P, HCH, ROWS2], BF16)
            for hc in range(HCH):
                acc = psb.tile([P, ROWS2], F32, tag="big")
                for c in range(CCH):
                    nc.tensor.matmul(
                        acc, wh_t[:, c, hc * P:(hc + 1) * P],
                        xln2T[:, c, :],
                        start=(c == 0), stop=(c == CCH - 1))
                nc.scalar.activation(hidT[:, hc, :], acc, AF.Relu,
                                     bias=bh_t[:, hc:hc + 1])
            mlpT = pers.tile([P, CCH, ROWS2], F32)
            for c in range(CCH):
                acc = psb.tile([P, ROWS2], F32, tag="big")
                for hc in range(HCH):
                    nc.tensor.matmul(
                        acc, wp_t[:, hc, c * P:(c + 1) * P],
                        hidT[:, hc, :],
                        start=(hc == 0), stop=(hc == HCH - 1))
                nc.scalar.activation(mlpT[:, c, :], acc, AF.Identity,
                                     bias=bp_t[:, c:c + 1])

            psb_cm.__exit__(None, None, None)
            pstf_cm = tc.tile_pool(name="pstf", bufs=2, space="PSUM")
            pstf = pstf_cm.__enter__()
            # transpose back to token-major, add residual, store
            out4 = pers.tile([P, NO, C], F32)
            for o in range(NO):
                for c in range(CCH):
                    tp = pstf.tile([P, P], F32, tag="tpf")
                    nc.tensor.transpose(tp, mlpT[:, c, o * P:(o + 1) * P],
                                        identf)
                    nc.vector.tensor_add(
                        out=out4[:, o, c * P:(c + 1) * P],
                        in0=tp, in1=xm[:, o, c * P:(c + 1) * P])
            pstf_cm.__exit__(None, None, None)
            oq_t = oq.rearrange("(o p) c -> p o c", p=P)
            for o in range(NO):
                nc.sync.dma_start(oq_t[:, o, :], out4[:, o, :])
    nc.compile()
    return nc


def _bf16(a):
    return np.ascontiguousarray(a.astype(ml_dtypes.bfloat16))


def kernel(x, ln1_g, ln1_b, wq, wk, wv, ln2_g, ln2_b, w_hidden, b_hidden,
           w_proj, b_proj):
    x = np.asarray(x, np.float32)
    ln1_g = np.asarray(ln1_g, np.float32)
    ln1_b = np.asarray(ln1_b, np.float32)
    wq = np.asarray(wq, np.float32)
    wk = np.asarray(wk, np.float32)
    wv = np.asarray(wv, np.float32)
    ln2_g = np.asarray(ln2_g, np.float32)
    ln2_b = np.asarray(ln2_b, np.float32)
    w_hidden = np.asarray(w_hidden, np.float32)
    b_hidden = np.asarray(b_hidden, np.float32)
    w_proj = np.asarray(w_proj, np.float32)
    b_proj = np.asarray(b_proj, np.float32)

    trace = bool(int(os.environ.get("KERNEL_TRACE", "0")))
    tkw = dict(trace=True, trace_cores=list(range(8))) if trace else {}

    # ---- fold LN1 gain into QKV weights; biases via LN1 shift ----
    wq_f = wq * ln1_g[None, :, None]
    wk_f = wk * ln1_g[None, :, None]
    wv_f = wv * ln1_g[None, :, None]
    bq_full = np.einsum("c,hcd->hd", ln1_b, wq)       # [H, D]
    bk_full = np.einsum("c,hcd->hd", ln1_b, wk)
    bv_full = np.einsum("c,hcd->hd", ln1_b, wv).reshape(C)

    if "k1" not in _cache:
        _cache["k1"] = build_kernel1()
    nc1 = _cache["k1"]

    in_maps1 = []
    for core in range(8):
        b, j = divmod(core, NC_PER_B)
        hs = slice(HG * j, HG * (j + 1))

        def wslice(w_f):
            return _bf16(w_f[hs].transpose(1, 0, 2).reshape(C, HG * D))

        def bias2(b_full):
            bs = b_full[hs].reshape(HG * D)
            out = np.zeros((P, 2), np.float32)
            out[:, 0] = bs[0:P]
            out[0:D, 1] = bs[P:P + D]
            return out

        in_maps1.append({
            "xb": np.ascontiguousarray(x[b]),
            "wq": wslice(wq_f), "wk": wslice(wk_f), "wv": wslice(wv_f),
            "bq": bias2(bq_full), "bk": bias2(bk_full),
        })
    r1 = bass_utils.run_bass_kernel_spmd(nc1, in_maps1,
                                         core_ids=list(range(8)), **tkw)

    attn = np.empty((B, T, H, D), np.float32)
    for core in range(8):
        b, j = divmod(core, NC_PER_B)
        attn[b, :, HG * j:HG * (j + 1), :] = \
            r1.results[core]["oO"].reshape(T, HG, D)
    a_flat = (attn.reshape(B, T, C) + bv_full[None, None, :]) \
        .reshape(B * T, C)
    x_flat = x.reshape(B * T, C)

    # ---- launch 2: LN2 + MLP, token-sharded ----
    wh_f = _bf16(w_hidden * ln2_g[:, None])
    bh_full = ln2_b @ w_hidden + b_hidden
    wp_c = _bf16(w_proj)
    bh_t = np.ascontiguousarray(bh_full.reshape(HCH, P).T.astype(np.float32))
    bp_t = np.ascontiguousarray(b_proj.reshape(CCH, P).T.astype(np.float32))

    if "k2" not in _cache:
        _cache["k2"] = build_kernel2()
    nc2 = _cache["k2"]

    in_maps2 = []
    for core in range(8):
        rows = slice(core * ROWS2, (core + 1) * ROWS2)
        in_maps2.append({
            "xq": np.ascontiguousarray(x_flat[rows]),
            "aq": np.ascontiguousarray(a_flat[rows]),
            "wh": wh_f, "wp": wp_c, "bh": bh_t, "bp": bp_t,
        })
    r2 = bass_utils.run_bass_kernel_spmd(nc2, in_maps2,
                                         core_ids=list(range(8)), **tkw)

    out = np.concatenate([r2.results[c]["oq"] for c in range(8)], axis=0)
    if trace:
        _cache["timings"] = [r1.exec_time_ns, r2.exec_time_ns]
        _cache["results"] = [r1, r2]
    return out.reshape(B, T, C)


# revision 34
# speedup vs baseline: 1.0012x; 1.0012x over previous
"""Trainium2 Bass kernel for a dense pre-LN transformer block.

Shapes (hardcoded): B=2, T=2048, C=768, H=12, D=64, hidden=3072, fp32 I/O.

Strategy (8 NeuronCores, two SPMD launches, no collectives):
  Launch 1 (attention): core = (batch b in {0,1}) x (head-group of 3 heads).
    Each core: LN1 (gain/bias folded into weights on host) -> transpose
    activations to feature-major bf16 -> Q/K/V projections for its 3 heads
    -> causal attention computed in S^T = K @ Q^T layout (keys on
    partitions, so the softmax matrix is consumed by the A@V matmul as the
    stationary operand directly -- no transposes of the 2048x2048 matrix).
    Softmax uses no max-subtraction (scores ~ N(0, 0.3); exp is safe) and
    gets its denominator for free from a ones-column appended to V.
    Output: per-head attention output [T, 3*64], token-major.
  Host glue: assemble heads, add V-bias term (softmax weights sum to 1, so
    the LN1-bias-through-Wv term is a per-feature constant), residual add.
  Launch 2 (MLP): core = 512 contiguous tokens of the flattened [4096, C].
    Each core: LN2 (folded) -> transpose -> MLP1 (bf16, relu+bias on
    ScalarE) -> MLP2 -> transpose back -> residual -> output rows.

All matmuls run in bf16 (4x faster than fp32 on the PE; fp32 accumulate in
PSUM). Weights are cast/folded on the host.
"""

import os
import sys
import math

for _p in ("/opt/trn_rl_repo", "/root/.axon_site/_ro/trn_rl_repo"):
    if _p not in sys.path and os.path.isdir(_p):
        sys.path.insert(0, _p)

import numpy as np
import ml_dtypes

import concourse.bass as bass
import concourse.mybir as mybir
import concourse.tile as tile
from concourse import bacc
from concourse import bass_utils
from concourse.masks import make_identity

BF16 = mybir.dt.bfloat16
F32 = mybir.dt.float32
AF = mybir.ActivationFunctionType

B, T, C, H, D = 2, 2048, 768, 12, 64
HID = 4 * C                     # 3072
EPS = 1e-5
SCALE = 1.0 / math.sqrt(C)      # reference scales scores by 1/sqrt(C)
NC_PER_B = 4                    # cores per batch in launch 1
HG = H // NC_PER_B              # heads per core (3)
P = 128
CCH = C // P                    # 6 feature chunks
TBLK = T // P                   # 16 token blocks of 128
NQB = T // 512                  # 4 q-blocks of 512
ROWS2 = (B * T) // 8            # 512 tokens per core in launch 2
HCH = HID // P                  # 24 hidden chunks

_cache = {}


def _ln_block(nc, pool, x_blk, eps_t, rows=P):
    """bn stats over free dim (C=768 via 3x256 subgroups) -> (mean, rstd)."""
    xg = x_blk.rearrange("p (s f) -> p s f", f=256)
    stats = pool.tile([P, 3, 6], F32, tag="ln_stats")
    for s in range(3):
        nc.vector.bn_stats(out=stats[:rows, s, :], in_=xg[:rows, s, :])
    mv = pool.tile([P, 2], F32, tag="ln_mv")
    nc.vector.bn_aggr(out=mv[:rows], in_=stats[:rows])
    rstd = pool.tile([P, 1], F32, tag="ln_rstd")
    nc.scalar.activation(rstd[:rows], mv[:rows, 1:2], AF.Sqrt,
                         bias=eps_t[:rows])
    nc.vector.reciprocal(rstd[:rows], rstd[:rows])
    return mv[:, 0:1], rstd


def build_kernel1():
    """LN1 + QKV (3 heads) + causal attention. One program, SPMD over 8."""
    nc = bacc.Bacc("TRN2", target_bir_lowering=False, debug=False,
                   num_devices=8)
    xb = nc.dram_tensor("xb", [T, C], F32, kind="ExternalInput")
    wq = nc.dram_tensor("wq", [C, HG * D], BF16, kind="ExternalInput")
    wk = nc.dram_tensor("wk", [C, HG * D], BF16, kind="ExternalInput")
    wv = nc.dram_tensor("wv", [C, HG * D], BF16, kind="ExternalInput")
    bq = nc.dram_tensor("bq", [P, 2], F32, kind="ExternalInput")
    bk = nc.dram_tensor("bk", [P, 2], F32, kind="ExternalInput")
    oO = nc.dram_tensor("oO", [T, HG * D], F32, kind="ExternalOutput")

    with tile.TileContext(nc) as tc:
        with (
            tc.tile_pool(name="persist", bufs=1) as pers,
            tc.tile_pool(name="stream", bufs=3) as stream,
            tc.tile_pool(name="small", bufs=4) as small,
            tc.tile_pool(name="pp", bufs=26) as pp,
            tc.tile_pool(name="psb", bufs=1, space="PSUM") as psb,
            tc.tile_pool(name="pss", bufs=4, space="PSUM") as pss,
            tc.tile_pool(name="pst", bufs=2, space="PSUM") as pst,
            tc.tile_pool(name="pso", bufs=1, space="PSUM") as pso,
        ):
            ident = pers.tile([P, P], BF16)
            make_identity(nc, ident)
            # causal mask for a diagonal 128x128 block of S^T[k, q]:
            # keep where q >= k
            mdiag = pers.tile([P, P], BF16)
            nc.gpsimd.memset(mdiag, 1.0)
            nc.gpsimd.affine_select(
                out=mdiag, in_=mdiag, compare_op=mybir.AluOpType.is_ge,
                fill=0.0, base=0, pattern=[[1, P]], channel_multiplier=-1)

            eps_t = pers.tile([P, 1], F32)
            nc.vector.memset(eps_t, EPS)

            wq_t = pers.tile([P, CCH, HG * D], BF16)
            wk_t = pers.tile([P, CCH, HG * D], BF16)
            wv_t = pers.tile([P, CCH, HG * D], BF16)
            nc.sync.dma_start(wq_t, wq.rearrange("(c p) d -> p c d", p=P))
            nc.sync.dma_start(wk_t, wk.rearrange("(c p) d -> p c d", p=P))
            nc.sync.dma_start(wv_t, wv.rearrange("(c p) d -> p c d", p=P))
            bq_t = pers.tile([P, 2], F32)
            bk_t = pers.tile([P, 2], F32)
            nc.sync.dma_start(bq_t, bq[:, :])
            nc.sync.dma_start(bk_t, bk[:, :])

            xhatT_l = [pers.tile([P, CCH, 512], BF16, name=f"xhT_{t}")
                       for t in range(T // 512)]
            x_tiled = xb.rearrange("(o p) c -> p o c", p=P)
            # ---- LN1 + transpose to feature-major ----
            for o in range(TBLK):
                x_blk = stream.tile([P, C], F32, tag="x_blk")
                nc.sync.dma_start(x_blk, x_tiled[:, o, :])
                mean, rstd = _ln_block(nc, small, x_blk, eps_t)
                xhat = stream.tile([P, C], BF16, tag="xhat")
                nc.vector.tensor_scalar(
                    out=xhat, in0=x_blk, scalar1=mean, scalar2=rstd,
                    op0=mybir.AluOpType.subtract, op1=mybir.AluOpType.mult)
                for c in range(CCH):
                    tp = pst.tile([P, P], BF16, tag="tp")
                    nc.tensor.transpose(tp, xhat[:, c * P:(c + 1) * P], ident)
                    nc.any.tensor_copy(
                        xhatT_l[o // 4][:, c, (o % 4) * P:(o % 4 + 1) * P],
                        tp)

            # ---- per 512-token chunk: QK proj, V proj, then attention for
            # q-block tch (causality: q-block tch only needs K/V <= tch), so
            # the PE projection work and ScalarE exp work interleave ----
            QT_l = [pers.tile([P, 2, 512], BF16, name=f"qt_{t}")
                    for t in range(T // 512)]
            KT_l = [pers.tile([P, 2, 512], BF16, name=f"kt_{t}")
                    for t in range(T // 512)]
            vaug_l = [pers.tile([P, HG, D + 1], BF16, name=f"va_{o}")
                      for o in range(TBLK)]
            oout = pers.tile([P, TBLK, HG, D], F32)
            for tch in range(T // 512):
                for dst_l, w_t, b_t in ((QT_l, wq_t, bq_t), (KT_l, wk_t, bk_t)):
                    for slot in range(2):
                        pr = P if slot == 0 else D  # partitions used
                        acc = psb.tile([P, 512], F32, tag="big")
                        for c in range(CCH):
                            nc.tensor.matmul(
                                acc[:pr],
                                w_t[:, c, slot * P: slot * P + pr],
                                xhatT_l[tch][:, c, :],
                                start=(c == 0), stop=(c == CCH - 1))
                        nc.vector.tensor_scalar_add(
                            dst_l[tch][:pr, slot, :],
                            acc[:pr], b_t[:pr, slot:slot + 1])
                for o in range(4 * tch, 4 * tch + 4):
                    nc.gpsimd.memset(vaug_l[o][:, :, D:D + 1], 1.0)
                    acc = psb.tile([P, 512], F32, tag="big")
                    for c in range(CCH):
                        nc.tensor.matmul(
                            acc[:, :HG * D],
                            xhatT_l[o // 4][:, c, (o % 4) * P:(o % 4 + 1) * P],
                            wv_t[:, c, :],
                            start=(c == 0), stop=(c == CCH - 1))
                    nc.vector.tensor_copy(
                        vaug_l[o][:, :, 0:D],
                        acc[:, :HG * D].rearrange("p (h d) -> p h d", h=HG))
                qb = tch
                for h in range(HG):
                    hslot = 0 if h < 2 else 1
                    hbase = D if h == 1 else 0
                    ptiles = {}
                    for kb in range(4 * qb + 4):
                        qs_rel = max(0, kb - 4 * qb) * P
                        n = 512 - qs_rel
                        sc = pss.tile([P, 512], F32, tag="sc")
                        nc.tensor.matmul(
                            sc[:, :n],
                            KT_l[kb // 4][hbase:hbase + D, hslot,
                                          (kb % 4) * P:(kb % 4 + 1) * P],
                            QT_l[qb][hbase:hbase + D, hslot, qs_rel:],
                            start=True, stop=True)
                        pt = pp.tile([P, 512], BF16, tag="p")
                        nc.scalar.activation(pt[:, :n], sc[:, :n], AF.Exp,
                                             scale=SCALE)
                        if kb >= 4 * qb:  # diagonal block: triangular mask
                            nc.vector.tensor_mul(pt[:, 0:P], pt[:, 0:P],
                                                 mdiag)
                        ptiles[kb] = (pt, qs_rel)
                    for s in range(4):
                        g = 4 * qb + s          # global 128-token q index
                        oacc = pso.tile([P, D + 1], F32, tag="oav")
                        for kb in range(g + 1):
                            pt, qs_rel = ptiles[kb]
                            off = s * P - qs_rel
                            nc.tensor.matmul(
                                oacc, pt[:, off:off + P],
                                vaug_l[kb][:, h, :],
                                start=(kb == 0), stop=(kb == g))
                        rec = small.tile([P, 1], F32, tag="rec")
                        nc.vector.reciprocal(rec, oacc[:, D:D + 1])
                        nc.vector.tensor_scalar_mul(
                            oout[:, g, h, :], oacc[:, 0:D], rec)

            nc.sync.dma_start(
                oO.rearrange("(o p) (h d) -> p o h d", p=P, h=HG), oout)
    nc.compile()
    return nc


def build_kernel2():
    """x_mid = x + attn; LN2 + MLP + residual for 512 tokens per core."""
    nc = bacc.Bacc("TRN2", target_bir_lowering=False, debug=False,
                   num_devices=8)
    xq = nc.dram_tensor("xq", [ROWS2, C], F32, kind="ExternalInput")
    aq = nc.dram_tensor("aq", [ROWS2, C], F32, kind="ExternalInput")
    wh = nc.dram_tensor("wh", [C, HID], BF16, kind="ExternalInput")
    wp = nc.dram_tensor("wp", [HID, C], BF16, kind="ExternalInput")
    bh = nc.dram_tensor("bh", [P, HCH], F32, kind="ExternalInput")
    bp = nc.dram_tensor("bp", [P, CCH], F32, kind="ExternalInput")
    oq = nc.dram_tensor("oq", [ROWS2, C], F32, kind="ExternalOutput")

    NO = ROWS2 // P  # 4 token sub-blocks
    with tile.TileContext(nc) as tc:
        with (
            tc.tile_pool(name="persist", bufs=1) as pers,
            tc.tile_pool(name="stream", bufs=3) as stream,
            tc.tile_pool(name="small", bufs=4) as small,
        ):
            eps_t = pers.tile([P, 1], F32)
            nc.vector.memset(eps_t, EPS)
            identb = pers.tile([P, P], BF16)
            make_identity(nc, identb)
            identf = pers.tile([P, P], F32)
            make_identity(nc, identf)

            xm = pers.tile([P, NO, C], F32)        # x_mid, token-major
            x4 = pers.tile([P, NO, C], F32)
            nc.sync.dma_start(x4, xq.rearrange("(o p) c -> p o c", p=P))
            a4 = pers.tile([P, NO, C], F32)
            nc.sync.dma_start(a4, aq.rearrange("(o p) c -> p o c", p=P))
            nc.vector.tensor_add(out=xm, in0=x4, in1=a4)

            wh_t = pers.tile([P, CCH, HID], BF16)
            wp_t = pers.tile([P, HCH, C], BF16)
            wh_r = wh.rearrange("(c p) n -> p c n", p=P)
            for g in range(4):
                nc.sync.dma_start(wh_t[:, :, g * 768:(g + 1) * 768],
                                  wh_r[:, :, g * 768:(g + 1) * 768])
            wp_r = wp.rearrange("(c p) n -> p c n", p=P)
            nc.scalar.dma_start(wp_t[:, :HCH // 2], wp_r[:, :HCH // 2])
            nc.scalar.dma_start(wp_t[:, HCH // 2:], wp_r[:, HCH // 2:])
            bh_t = pers.tile([P, HCH], F32)
            bp_t = pers.tile([P, CCH], F32)
            nc.scalar.dma_start(bh_t, bh[:, :])
            nc.scalar.dma_start(bp_t, bp[:, :])

            # LN2 + transpose -> feature-major bf16 [C, 512]
            xln2T = pers.tile([P, CCH, ROWS2], BF16)
            pst_cm = tc.tile_pool(name="pst", bufs=2, space="PSUM")
            pst = pst_cm.__enter__()
            for o in range(NO):
                mean, rstd = _ln_block(nc, small, xm[:, o, :], eps_t)
                xln = stream.tile([P, C], BF16, tag="xln")
                nc.vector.tensor_scalar(
                    out=xln, in0=xm[:, o, :], scalar1=mean, scalar2=rstd,
                    op0=mybir.AluOpType.subtract, op1=mybir.AluOpType.mult)
                for c in range(CCH):
                    tp = pst.tile([P, P], BF16, tag="tpb")
                    nc.tensor.transpose(tp, xln[:, c * P:(c + 1) * P], identb)
                    nc.vector.tensor_copy(xln2T[:, c, o * P:(o + 1) * P], tp)

            pst_cm.__exit__(None, None, None)
            psb_cm = tc.tile_pool(name="psb", bufs=3, space="PSUM")
            psb = psb_cm.__enter__()
            hidT = pers.tile([P, HCH, ROWS2], BF16)
            for hc in range(HCH):
                acc = psb.tile([P, ROWS2], F32, tag="big")
                for c in range(CCH):
                    nc.tensor.matmul(
                        acc, wh_t[:, c, hc * P:(hc + 1) * P],
                        xln2T[:, c, :],
                        start=(c == 0), stop=(c == CCH - 1))
                nc.scalar.activation(hidT[:, hc, :], acc, AF.Relu,
                                     bias=bh_t[:, hc:hc + 1])
            mlpT = pers.tile([P, CCH, ROWS2], F32)
            for c in range(CCH):
                acc = psb.tile([P, ROWS2], F32, tag="big")
                for hc in range(HCH):
                    nc.tensor.matmul(
                        acc, wp_t[:, hc, c * P:(c + 1) * P],
                        hidT[:, hc, :],
                        start=(hc == 0), stop=(hc == HCH - 1))
                nc.scalar.activation(mlpT[:, c, :], acc, AF.Identity,
                                     bias=bp_t[:, c:c + 1])

            psb_cm.__exit__(None, None, None)
            pstf_cm = tc.tile_pool(name="pstf", bufs=2, space="PSUM")
            pstf = pstf_cm.__enter__()
            # transpose back to token-major, add residual, store
            out4 = pers.tile([P, NO, C], F32)
            for o in range(NO):
                for c in range(CCH):
                    tp = pstf.tile([P, P], F32, tag="tpf")
                    nc.tensor.transpose(tp, mlpT[:, c, o * P:(o + 1) * P],
                                        identf)
                    nc.vector.tensor_add(
                        out=out4[:, o, c * P:(c + 1) * P],
                        in0=tp, in1=xm[:, o, c * P:(c + 1) * P])
            pstf_cm.__exit__(None, None, None)
            oq_t = oq.rearrange("(o p) c -> p o c", p=P)
            for o in range(NO):
                nc.sync.dma_start(oq_t[:, o, :], out4[:, o, :])
    nc.compile()
    return nc


def _bf16(a):
    return np.ascontiguousarray(a.astype(ml_dtypes.bfloat16))


def kernel(x, ln1_g, ln1_b, wq, wk, wv, ln2_g, ln2_b, w_hidden, b_hidden,
           w_proj, b_proj):
    x = np.asarray(x, np.float32)
    ln1_g = np.asarray(ln1_g, np.float32)
    ln1_b = np.asarray(ln1_b, np.float32)
    wq = np.asarray(wq, np.float32)
    wk = np.asarray(wk, np.float32)
    wv = np.asarray(wv, np.float32)
    ln2_g = np.asarray(ln2_g, np.float32)
    ln2_b = np.asarray(ln2_b, np.float32)
    w_hidden = np.asarray(w_hidden, np.float32)
    b_hidden = np.asarray(b_hidden, np.float32)
    w_proj = np.asarray(w_proj, np.float32)
    b_proj = np.asarray(b_proj, np.float32)

    trace = bool(int(os.environ.get("KERNEL_TRACE", "0")))
    tkw = dict(trace=True, trace_cores=list(range(8))) if trace else {}

    # ---- fold LN1 gain into QKV weights; biases via LN1 shift ----
    wq_f = wq * ln1_g[None, :, None]
    wk_f = wk * ln1_g[None, :, None]
    wv_f = wv * ln1_g[None, :, None]
    bq_full = np.einsum("c,hcd->hd", ln1_b, wq)       # [H, D]
    bk_full = np.einsum("c,hcd->hd", ln1_b, wk)
    bv_full = np.einsum("c,hcd->hd", ln1_b, wv).reshape(C)

    if "k1" not in _cache:
        _cache["k1"] = build_kernel1()
    nc1 = _cache["k1"]

    in_maps1 = []
    for core in range(8):
        b, j = divmod(core, NC_PER_B)
        hs = slice(HG * j, HG * (j + 1))

        def wslice(w_f):
            return _bf16(w_f[hs].transpose(1, 0, 2).reshape(C, HG * D))

        def bias2(b_full):
            bs = b_full[hs].reshape(HG * D)
            out = np.zeros((P, 2), np.float32)
            out[:, 0] = bs[0:P]
            out[0:D, 1] = bs[P:P + D]
            return out

        in_maps1.append({
            "xb": np.ascontiguousarray(x[b]),
            "wq": wslice(wq_f), "wk": wslice(wk_f), "wv": wslice(wv_f),
            "bq": bias2(bq_full), "bk": bias2(bk_full),
        })
    r1 = bass_utils.run_bass_kernel_spmd(nc1, in_maps1,
                                         core_ids=list(range(8)), **tkw)

    attn = np.empty((B, T, H, D), np.float32)
    for core in range(8):
        b, j = divmod(core, NC_PER_B)
        attn[b, :, HG * j:HG * (j + 1), :] = \
            r1.results[core]["oO"].reshape(T, HG, D)
    a_flat = (attn.reshape(B, T, C) + bv_full[None, None, :]) \
        .reshape(B * T, C)
    x_flat = x.reshape(B * T, C)

    # ---- launch 2: LN2 + MLP, token-sharded ----
    wh_f = _bf16(w_hidden * ln2_g[:, None])
    bh_full = ln2_b @ w_hidden + b_hidden
    wp_c = _bf16(w_proj)
    bh_t = np.ascontiguousarray(bh_full.reshape(HCH, P).T.astype(np.float32))
    bp_t = np.ascontiguousarray(b_proj.reshape(CCH, P).T.astype(np.float32))

    if "k2" not in _cache:
        _cache["k2"] = build_kernel2()
    nc2 = _cache["k2"]

    in_maps2 = []
    for core in range(8):
        rows = slice(core * ROWS2, (core + 1) * ROWS2)
        in_maps2.append({
            "xq": np.ascontiguousarray(x_flat[rows]),
            "aq": np.ascontiguousarray(a_flat[rows]),
            "wh": wh_f, "wp": wp_c, "bh": bh_t, "bp": bp_t,
        })
    r2 = bass_utils.run_bass_kernel_spmd(nc2, in_maps2,
                                         core_ids=list(range(8)), **tkw)

    out = np.concatenate([r2.results[c]["oq"] for c in range(8)], axis=0)
    if trace:
        _cache["timings"] = [r1.exec_time_ns, r2.exec_time_ns]
        _cache["results"] = [r1, r2]
    return out.reshape(B, T, C)
